# revision 12
# baseline (speedup 1.0000x reference)
"""Causal self-attention (B=4, S=2048, D=1024, fp32, single head) on 8 TRN2 cores.

Sharding: core c -> (batch b=c//2, parity p=c%2). Each core owns 8 of the 16
q-tiles (128 rows each) of its batch, chosen so causal work is balanced:
parity 0 -> global q-tiles [0,3,4,7,8,11,12,15], parity 1 -> [1,2,5,6,9,10,13,14].
Both parities run with padded kv-extents [2,4,6,8,10,12,14,16] (in sk-tiles)
per local q-tile so a single SPMD program serves all cores; causality/padding
is enforced by per-core mask *data* applied to the last two sk-tiles of each
q-tile.

Algebraic refactor (v2): the K and V projections are eliminated entirely.
With q = x Wq^T + bq, k = x Wk^T + bk, s = 1/sqrt(D):
    scores = (q k^T) s = x M x^T + a_i + b_j + c,   M = Wq^T Wk s (host-側)
where a_i (per query) and c cancel in softmax, and b_j = x_j . (Wk^T bq) s is
a tiny per-key bias folded into the exp activation. Since softmax rows sum
to 1:
    out = attn (x Wv^T + bv) Wp^T + bp = (attn x) W2^T + (bp + Wp bv),
    W2 = Wp Wv (host-side).
So the device computes only: z = x_own M (the "q"), scores^T = xT-tiles x z,
exp(+b_j), rowsum via ones-matmul, AV = E x (x natural-layout stationary),
proj = AV W2^T, normalize + bias. PE work drops from 17.7 to 9.1 GF/core.

All matmuls bf16 with fp32 PSUM accumulation; host pre-computes M, W2,
b_j, bp + Wp bv and pre-transposes/casts x.
"""

import numpy as np
import ml_dtypes
from contextlib import ExitStack

import concourse.bass as bass
import concourse.mybir as mybir
from concourse import bacc
from concourse.tile import TileContext
from concourse.bass_utils import run_bass_kernel_spmd

P = 128
D = 1024
S = 2048
B = 4
NCORES = 8
SQL = S // 2            # local q rows per core
NE = D // P             # 8 e-subtiles of the embedding dim
NSK = S // P            # 16 sk tiles
GT = [[0, 3, 4, 7, 8, 11, 12, 15], [1, 2, 5, 6, 9, 10, 13, 14]]
PADK = [2, 4, 6, 8, 10, 12, 14, 16]   # padded kv sk-tiles per local q-tile
CHUNKS = [(0, 8), (1, 16)]            # (sq-chunk idx, padded sk-tiles)
SCALE = 1.0 / 32.0                    # 1/sqrt(D)

bf16 = mybir.dt.bfloat16
f32 = mybir.dt.float32
nbf = ml_dtypes.bfloat16
AF = mybir.ActivationFunctionType


def _chunk_start(c, j):
    """First valid local sq-subtile (0..3) of chunk c for sk-tile j."""
    return sum(1 for i in range(4) if PADK[c * 4 + i] <= j)


def build_nc(repeat=1, psa=6, psb=2, xstb=3, epb=24, apb=2, opb=3,
             hint=False, srst=False, denom2=False, obat=False, outq=True):
    nc = bacc.Bacc("TRN2", target_bir_lowering=False, num_devices=NCORES)

    xT_h = nc.dram_tensor("xT", [D, S], bf16, kind="ExternalInput")
    xN_h = nc.dram_tensor("xN", [S, D], bf16, kind="ExternalInput")
    xqT_h = nc.dram_tensor("xqT", [D, SQL], bf16, kind="ExternalInput")
    mq_h = nc.dram_tensor("mQ", [D, D], bf16, kind="ExternalInput")
    w2_h = nc.dram_tensor("w2T", [D, D], bf16, kind="ExternalInput")
    bj_h = nc.dram_tensor("bjT", [P, NSK], f32, kind="ExternalInput")
    bp_h = nc.dram_tensor("bp_bc", [P, D], f32, kind="ExternalInput")
    mk_h = nc.dram_tensor("masks", [16, P, P], mybir.dt.uint8, kind="ExternalInput")
    out_h = nc.dram_tensor("out", [SQL, D], f32, kind="ExternalOutput")

    with TileContext(nc) as tc, ExitStack() as ctx:
        const = ctx.enter_context(tc.tile_pool(name="const", bufs=1))
        wpool = ctx.enter_context(tc.tile_pool(name="wpool", bufs=2))
        xst = ctx.enter_context(tc.tile_pool(name="xst", bufs=xstb))
        xtp = ctx.enter_context(tc.tile_pool(name="xtp", bufs=1))
        xnp = ctx.enter_context(tc.tile_pool(name="xnp", bufs=1))
        zpool = ctx.enter_context(tc.tile_pool(name="zpool", bufs=1))
        epool = ctx.enter_context(tc.tile_pool(name="epool", bufs=epb))
        apool = ctx.enter_context(tc.tile_pool(name="apool", bufs=apb))
        opool = ctx.enter_context(tc.tile_pool(name="opool", bufs=opb))
        rpool = ctx.enter_context(tc.tile_pool(name="rpool", bufs=2 if denom2 else 8))
        rbp = ctx.enter_context(tc.tile_pool(name="rbp", bufs=2)) if denom2 else None
        psA = ctx.enter_context(tc.tile_pool(name="psA", bufs=psa, space="PSUM"))
        psB = ctx.enter_context(tc.tile_pool(name="psB", bufs=psb, space="PSUM"))

        rep_cm = tc.For_i(0, repeat, 1, hint_engines=tuple(nc.engines) if hint else (), staggered_reset=srst) if repeat > 1 else None
        if rep_cm is not None:
            rep_cm.__enter__()
        for _rep in range(1):
            # constants (gpsimd DGE queue)
            bj_sb = const.tile([P, NSK], f32, name="bj_sb", tag="bj")
            nc.gpsimd.dma_start(bj_sb, bj_h[:])
            bp_sb = const.tile([P, D], f32, name="bp_sb", tag="bp")
            nc.gpsimd.dma_start(bp_sb, bp_h[:])
            mk_sb = const.tile([P, 16, P], mybir.dt.uint8, name="mk_sb", tag="mk")
            nc.gpsimd.dma_start(mk_sb, mk_h[:].rearrange("i p q -> p i q"))
            ones_col = const.tile([P, 1], bf16, name="ones_col", tag="ones")
            nc.vector.memset(ones_col, 1.0)
            zeros_pp = const.tile([P, P], bf16, name="zeros_pp", tag="zpp")
            nc.vector.memset(zeros_pp, 0.0)

            # persistent per-core tensors (bf16)
            xT_sb = xtp.tile([P, NE, S], bf16, name="xT_sb", tag="xT")
            xN_sb = xnp.tile([P, NSK, D], bf16, name="xN_sb", tag="xN")
            zT = zpool.tile([P, NE, SQL], bf16, name="zT_sb", tag="zT")

            # ---- z pass: zT[e, sq] = M-tiles (stationary) x xqT chunks ----
            # lead-in: interleave M row-blocks and xqT chunk-0 slices across
            # both HWDGE queues so the first psum chain starts ~1-2 us in.
            mq = wpool.tile([P, NE, D], bf16, name="mq", tag="w")
            xc0 = xst.tile([P, NE, 512], bf16, name="xcq0", tag="xt")
            xc1 = xst.tile([P, NE, 512], bf16, name="xcq1", tag="xt")
            for d_ in range(NE):
                nc.sync.dma_start(mq[:, d_, :], mq_h[d_ * P:(d_ + 1) * P, :])
                nc.scalar.dma_start(
                    xc0[:, d_, :], xqT_h[d_ * P:(d_ + 1) * P, 0:512]
                )
            for d_ in range(NE):
                nc.scalar.dma_start(
                    xc1[:, d_, :], xqT_h[d_ * P:(d_ + 1) * P, 512:1024]
                )
            # bulk loads behind the z lead-in, split across both queues in
            # need-order: xT S-half0 (scores c0 stationary, needed ~25us),
            # xT half1 (c1), then xN (AV, ~50us), then w2 (proj, ~90us).
            for h in range(2):
                for e in range(NE):
                    (nc.sync if e < 4 else nc.scalar).dma_start(
                        xT_sb[:, e, h * 1024:(h + 1) * 1024],
                        xT_h[e * P:(e + 1) * P, h * 1024:(h + 1) * 1024],
                    )
            for j in range(NSK):
                (nc.sync if j < 8 else nc.scalar).dma_start(
                    xN_sb[:, j, :], xN_h[j * P:(j + 1) * P, :]
                )
            # z matmuls d-outer over 4 open psums: the first matmul needs
            # only the d=0 slices, so PE starts ~1.5us in and streams as
            # DMAs land.
            for c in range(2):
                xc = xc0 if c == 0 else xc1
                for eh in range(2):
                    pss = [psA.tile([P, 512], f32, name=f"psq{c}_{eh}_{e4}", tag="psA")
                           for e4 in range(4)]
                    for d_ in range(NE):
                        for e4 in range(4):
                            e = eh * 4 + e4
                            nc.tensor.matmul(
                                pss[e4], mq[:, d_, e * P:(e + 1) * P], xc[:, d_, :],
                                start=(d_ == 0), stop=(d_ == NE - 1),
                            )
                    for e4 in range(4):
                        e = eh * 4 + e4
                        nc.vector.tensor_copy(
                            zT[:, e, c * 512:(c + 1) * 512], pss[e4]
                        )

            # proj weights (needed last; scalar queue keeps sync free for
            # the output stores)
            w2 = wpool.tile([P, NE, D], bf16, name="w2", tag="w")
            for m in range(NE):
                nc.scalar.dma_start(w2[:, m, :], w2_h[m * P:(m + 1) * P, :])

            # ---- attention per sq-chunk ----
            for c, Kc in CHUNKS:
                # scoresT[j] = xT-tiles (stationary) x zT slice; exp(+b_j) -> bf16
                exps = []
                for j in range(Kc):
                    s_off = _chunk_start(c, j) * P
                    Nj = 512 - s_off
                    ps = psA.tile([P, 512], f32, name=f"pss{c}_{j}", tag="psA")
                    psv = ps[:, :Nj]
                    for e in range(NE):
                        nc.tensor.matmul(
                            psv, xT_sb[:, e, j * P:(j + 1) * P],
                            zT[:, e, c * 512 + s_off:(c + 1) * 512],
                            start=(e == 0), stop=(e == NE - 1),
                        )
                    ex = epool.tile([P, 512], bf16, name=f"exp{c}_{j}", tag="exp")
                    exv = ex[:, :Nj]
                    nc.scalar.activation(
                        exv, psv, AF.Exp, bias=bj_sb[:, j:j + 1], scale=1.0
                    )
                    # mask the first sq-subtile where this j is diagonal/padding
                    mi = [i for i in range(4)
                          if PADK[c * 4 + i] - 2 == j or PADK[c * 4 + i] - 1 == j]
                    if mi:
                        i = mi[0]
                        which = int(PADK[c * 4 + i] - 1 == j)
                        gidx = 2 * (c * 4 + i) + which
                        nc.vector.copy_predicated(
                            ex[:, :P], mk_sb[:, gidx, :], zeros_pp
                        )
                    exps.append((exv, s_off, Nj))

                if denom2:
                    # denom[1, sq]: ones (stationary) x exp tiles, j-accumulated
                    # in one PSUM row; then 1/x, consumed partition-broadcast
                    # (stride-0) by the AV evacuation tensor_tensor.
                    pd = psB.tile([1, 512], f32, name=f"dn{c}", tag="psB")
                    for j in range(Kc):
                        exv, s_off, _ = exps[j]
                        nc.tensor.matmul(
                            pd[:, s_off:512], ones_col, exv,
                            start=(j == 0), stop=(j == Kc - 1),
                        )
                    rc = rpool.tile([1, 512], f32, name=f"rc{c}", tag="rc")
                    nc.vector.reciprocal(rc, pd)
                    rb = rbp.tile([P, 512], f32, name=f"rb{c}", tag="rb")
                    nc.gpsimd.partition_broadcast(rb, rc)
                    recips = None
                else:
                    # denom^T[sq,1] per sq-subtile: exp-tiles (stationary) x ones
                    recips = []
                    for s4 in range(4):
                        Pi = PADK[c * 4 + s4]
                        pd = psB.tile([P, 1], f32, name=f"dn{c}_{s4}", tag="psB")
                        for j in range(Pi):
                            exv, s_off, _ = exps[j]
                            nc.tensor.matmul(
                                pd, exv[:, s4 * P - s_off:(s4 + 1) * P - s_off], ones_col,
                                start=(j == 0), stop=(j == Pi - 1),
                            )
                        rc = rpool.tile([P, 1], f32, name=f"rc{c}_{s4}", tag="rc")
                        nc.vector.reciprocal(rc, pd)
                        recips.append(rc)

                # AV: aoT[e, sq] accumulated over sk-tiles; xN tiles stationary
                ao = apool.tile([P, NE, 512], bf16, name=f"ao{c}", tag="ao")
                for m in range(NE):
                    pa = psA.tile([P, 512], f32, name=f"pa{c}_{m}", tag="psA")
                    for j in range(Kc):
                        exv, s_off, _ = exps[j]
                        nc.tensor.matmul(
                            pa[:, s_off:512], xN_sb[:, j, m * P:(m + 1) * P], exv,
                            start=(j == 0), stop=(j == Kc - 1),
                        )
                    if denom2:
                        nc.vector.tensor_tensor(
                            ao[:, m, :], pa, rb, op=mybir.AluOpType.mult
                        )
                    else:
                        nc.vector.tensor_copy(ao[:, m, :], pa)

                # proj: out[sq, e'] = aoT-tiles (stationary) x W2^T; normalize+bias
                for s4 in range(4):
                    otw = (opool.tile([P, D], f32, name=f"otw{c}_{s4}", tag="ot")
                           if obat else None)
                    for n in range(2):
                        po = psA.tile([P, 512], f32, name=f"po{c}_{s4}_{n}", tag="psA")
                        for m in range(NE):
                            nc.tensor.matmul(
                                po, ao[:, m, s4 * P:(s4 + 1) * P],
                                w2[:, m, n * 512:(n + 1) * 512],
                                start=(m == 0), stop=(m == NE - 1),
                            )
                        ot = (otw[:, n * 512:(n + 1) * 512] if obat else
                              opool.tile([P, 512], f32, name=f"ot{c}_{s4}_{n}", tag="ot"))
                        if denom2:
                            nc.vector.tensor_tensor(
                                ot, po, bp_sb[:, n * 512:(n + 1) * 512],
                                op=mybir.AluOpType.add,
                            )
                        else:
                            nc.vector.scalar_tensor_tensor(
                                ot, po, recips[s4], bp_sb[:, n * 512:(n + 1) * 512],
                                op0=mybir.AluOpType.mult, op1=mybir.AluOpType.add,
                            )
                        if not obat:
                            oeng = (nc.scalar if (outq and (s4 * 2 + n) % 2)
                                    else nc.sync)
                            oeng.dma_start(
                                out_h[c * 512 + s4 * P:c * 512 + (s4 + 1) * P,
                                      n * 512:(n + 1) * 512],
                                ot,
                            )
                    if obat:
                        nc.sync.dma_start(
                            out_h[c * 512 + s4 * P:c * 512 + (s4 + 1) * P, :], otw
                        )
        if rep_cm is not None:
            rep_cm.__exit__(None, None, None)
    nc.finalize()
    return nc


_NC_CACHE = None


def _get_nc():
    global _NC_CACHE
    if _NC_CACHE is None:
        _NC_CACHE = build_nc()
    return _NC_CACHE


def _prep_inputs(x, Wq, bq, Wk, bk, Wv, bv, Wp, bp):
    """Host-side shard prep: returns list of per-core input dicts."""
    x = np.asarray(x, np.float32)
    Wq, bq = np.asarray(Wq, np.float32), np.asarray(bq, np.float32)
    Wk, bk = np.asarray(Wk, np.float32), np.asarray(bk, np.float32)
    Wv, bv = np.asarray(Wv, np.float32), np.asarray(bv, np.float32)
    Wp, bp = np.asarray(Wp, np.float32), np.asarray(bp, np.float32)

    u = (Wk.T @ bq) * SCALE                      # per-key bias direction
    shared = {
        "mQ": np.ascontiguousarray((Wq.T @ Wk) * SCALE).astype(nbf),
        "w2T": np.ascontiguousarray((Wp @ Wv).T).astype(nbf),
        "bp_bc": np.ascontiguousarray(
            np.tile((bp + Wp @ bv).astype(np.float32), (P, 1))
        ),
    }

    kk = np.arange(P)[:, None]
    qq = np.arange(P)[None, :]
    in_maps = []
    per_batch = {}
    for b in range(B):
        xb = x[b]
        per_batch[b] = {
            "xT": np.ascontiguousarray(xb.T).astype(nbf),
            "xN": np.ascontiguousarray(xb).astype(nbf),
            "bjT": np.ascontiguousarray((xb @ u).reshape(NSK, P).T.astype(np.float32)),
        }
    for c in range(NCORES):
        b, p = divmod(c, 2)
        g = GT[p]
        xb = x[b]
        qrows = np.concatenate([xb[t * P:(t + 1) * P] for t in g], 0)
        xqT = np.ascontiguousarray(qrows.T).astype(nbf)
        masks = np.zeros((16, P, P), np.float32)
        for i in range(8):
            Pi, gi = PADK[i], g[i]
            for w, j in ((0, Pi - 2), (1, Pi - 1)):
                # 1.0 where DISALLOWED (key index > query index)
                masks[2 * i + w] = ((j * P + kk) > (gi * P + qq)).astype(np.float32)
        in_maps.append({
            "xqT": xqT, "masks": masks.astype(np.uint8),
            **per_batch[b], **shared,
        })
    return in_maps


def _scatter_outputs(results):
    out = np.empty((B, S, D), np.float32)
    for c in range(NCORES):
        b, p = divmod(c, 2)
        o = results[c]["out"]
        for i, t in enumerate(GT[p]):
            out[b, t * P:(t + 1) * P] = o[i * P:(i + 1) * P]
    return out


def run(inputs, trace=False):
    nc = _get_nc()
    in_maps = _prep_inputs(**inputs)
    res = run_bass_kernel_spmd(
        nc, in_maps, core_ids=list(range(NCORES)), trace=trace
    )
    return _scatter_outputs(res.results), res


def kernel(**inputs):
    out, _ = run(inputs)
    return out


# revision 14
# speedup vs baseline: 1.0055x; 1.0055x over previous
"""Causal self-attention (B=4, S=2048, D=1024, fp32, single head) on 8 TRN2 cores.

Sharding: core c -> (batch b=c//2, parity p=c%2). Each core owns 8 of the 16
q-tiles (128 rows each) of its batch, chosen so causal work is balanced:
parity 0 -> global q-tiles [0,3,4,7,8,11,12,15], parity 1 -> [1,2,5,6,9,10,13,14].
Both parities run with padded kv-extents [2,4,6,8,10,12,14,16] (in sk-tiles)
per local q-tile so a single SPMD program serves all cores; causality/padding
is enforced by per-core mask *data* applied to the last two sk-tiles of each
q-tile.

Algebraic refactor (v2): the K and V projections are eliminated entirely.
With q = x Wq^T + bq, k = x Wk^T + bk, s = 1/sqrt(D):
    scores = (q k^T) s = x M x^T + a_i + b_j + c,   M = Wq^T Wk s (host-側)
where a_i (per query) and c cancel in softmax, and b_j = x_j . (Wk^T bq) s is
a tiny per-key bias folded into the exp activation. Since softmax rows sum
to 1:
    out = attn (x Wv^T + bv) Wp^T + bp = (attn x) W2^T + (bp + Wp bv),
    W2 = Wp Wv (host-side).
So the device computes only: z = x_own M (the "q"), scores^T = xT-tiles x z,
exp(+b_j), rowsum via ones-matmul, AV = E x (x natural-layout stationary),
proj = AV W2^T, normalize + bias. PE work drops from 17.7 to 9.1 GF/core.

All matmuls bf16 with fp32 PSUM accumulation; host pre-computes M, W2,
b_j, bp + Wp bv and pre-transposes/casts x.
"""

import numpy as np
import ml_dtypes
from contextlib import ExitStack

import concourse.bass as bass
import concourse.mybir as mybir
from concourse import bacc
from concourse.tile import TileContext
from concourse.bass_utils import run_bass_kernel_spmd

P = 128
D = 1024
S = 2048
B = 4
NCORES = 8
SQL = S // 2            # local q rows per core
NE = D // P             # 8 e-subtiles of the embedding dim
NSK = S // P            # 16 sk tiles
GT = [[0, 3, 4, 7, 8, 11, 12, 15], [1, 2, 5, 6, 9, 10, 13, 14]]
PADK = [2, 4, 6, 8, 10, 12, 14, 16]   # padded kv sk-tiles per local q-tile
CHUNKS = [(0, 8), (1, 16)]            # (sq-chunk idx, padded sk-tiles)
SCALE = 1.0 / 32.0                    # 1/sqrt(D)

bf16 = mybir.dt.bfloat16
f32 = mybir.dt.float32
nbf = ml_dtypes.bfloat16
AF = mybir.ActivationFunctionType


def _chunk_start(c, j):
    """First valid local sq-subtile (0..3) of chunk c for sk-tile j."""
    return sum(1 for i in range(4) if PADK[c * 4 + i] <= j)


def build_nc(repeat=1, psa=6, psb=2, xstb=3, epb=24, apb=2, opb=3,
             hint=False, srst=False, denom2=False, obat=False, outq=True):
    nc = bacc.Bacc("TRN2", target_bir_lowering=False, num_devices=NCORES)

    xT_h = nc.dram_tensor("xT", [D, S], bf16, kind="ExternalInput")
    xN_h = nc.dram_tensor("xN", [S, D], bf16, kind="ExternalInput")
    xqT_h = nc.dram_tensor("xqT", [D, SQL], bf16, kind="ExternalInput")
    mq_h = nc.dram_tensor("mQ", [D, D], bf16, kind="ExternalInput")
    w2_h = nc.dram_tensor("w2T", [D, D], bf16, kind="ExternalInput")
    bj_h = nc.dram_tensor("bjT", [P, NSK], f32, kind="ExternalInput")
    bp_h = nc.dram_tensor("bp_bc", [P, D], f32, kind="ExternalInput")
    mk_h = nc.dram_tensor("masks", [16, P, P], mybir.dt.uint8, kind="ExternalInput")
    out_h = nc.dram_tensor("out", [SQL, D], f32, kind="ExternalOutput")

    with TileContext(nc) as tc, ExitStack() as ctx:
        const = ctx.enter_context(tc.tile_pool(name="const", bufs=1))
        wpool = ctx.enter_context(tc.tile_pool(name="wpool", bufs=2))
        xst = ctx.enter_context(tc.tile_pool(name="xst", bufs=xstb))
        xtp = ctx.enter_context(tc.tile_pool(name="xtp", bufs=1))
        xnp = ctx.enter_context(tc.tile_pool(name="xnp", bufs=1))
        zpool = ctx.enter_context(tc.tile_pool(name="zpool", bufs=1))
        epool = ctx.enter_context(tc.tile_pool(name="epool", bufs=epb))
        apool = ctx.enter_context(tc.tile_pool(name="apool", bufs=apb))
        opool = ctx.enter_context(tc.tile_pool(name="opool", bufs=opb))
        rpool = ctx.enter_context(tc.tile_pool(name="rpool", bufs=2 if denom2 else 8))
        rbp = ctx.enter_context(tc.tile_pool(name="rbp", bufs=2)) if denom2 else None
        psA = ctx.enter_context(tc.tile_pool(name="psA", bufs=psa, space="PSUM"))
        psB = ctx.enter_context(tc.tile_pool(name="psB", bufs=psb, space="PSUM"))

        rep_cm = tc.For_i(0, repeat, 1, hint_engines=tuple(nc.engines) if hint else (), staggered_reset=srst) if repeat > 1 else None
        if rep_cm is not None:
            rep_cm.__enter__()
        for _rep in range(1):
            # constants (gpsimd DGE queue)
            bj_sb = const.tile([P, NSK], f32, name="bj_sb", tag="bj")
            nc.gpsimd.dma_start(bj_sb, bj_h[:])
            bp_sb = const.tile([P, D], f32, name="bp_sb", tag="bp")
            nc.gpsimd.dma_start(bp_sb, bp_h[:])
            mk_sb = const.tile([P, 16, P], mybir.dt.uint8, name="mk_sb", tag="mk")
            nc.gpsimd.dma_start(mk_sb, mk_h[:].rearrange("i p q -> p i q"))
            ones_col = const.tile([P, 1], bf16, name="ones_col", tag="ones")
            nc.vector.memset(ones_col, 1.0)
            zeros_pp = const.tile([P, P], bf16, name="zeros_pp", tag="zpp")
            nc.vector.memset(zeros_pp, 0.0)

            # persistent per-core tensors (bf16)
            xT_sb = xtp.tile([P, NE, S], bf16, name="xT_sb", tag="xT")
            xN_sb = xnp.tile([P, NSK, D], bf16, name="xN_sb", tag="xN")
            zT = zpool.tile([P, NE, SQL], bf16, name="zT_sb", tag="zT")

            # ---- z pass: zT[e, sq] = M-tiles (stationary) x xqT chunks ----
            # lead-in: interleave M row-blocks and xqT chunk-0 slices across
            # both HWDGE queues so the first psum chain starts ~1-2 us in.
            mq = wpool.tile([P, NE, D], bf16, name="mq", tag="w")
            xc0 = xst.tile([P, NE, 512], bf16, name="xcq0", tag="xt")
            xc1 = xst.tile([P, NE, 512], bf16, name="xcq1", tag="xt")
            for d_ in range(NE):
                nc.sync.dma_start(mq[:, d_, :], mq_h[d_ * P:(d_ + 1) * P, :])
                nc.scalar.dma_start(
                    xc0[:, d_, :], xqT_h[d_ * P:(d_ + 1) * P, 0:512]
                )
            for d_ in range(NE):
                nc.scalar.dma_start(
                    xc1[:, d_, :], xqT_h[d_ * P:(d_ + 1) * P, 512:1024]
                )
            # bulk loads behind the z lead-in, split across both queues in
            # need-order: xT S-half0 (scores c0 stationary, needed ~25us),
            # xT half1 (c1), then xN (AV, ~50us), then w2 (proj, ~90us).
            for h in range(2):
                for e in range(NE):
                    (nc.sync if e < 4 else nc.scalar).dma_start(
                        xT_sb[:, e, h * 1024:(h + 1) * 1024],
                        xT_h[e * P:(e + 1) * P, h * 1024:(h + 1) * 1024],
                    )
            for j in range(NSK):
                (nc.sync if j < 8 else nc.scalar).dma_start(
                    xN_sb[:, j, :], xN_h[j * P:(j + 1) * P, :]
                )
            # z matmuls d-outer over 4 open psums: the first matmul needs
            # only the d=0 slices, so PE starts ~1.5us in and streams as
            # DMAs land.
            for c in range(2):
                xc = xc0 if c == 0 else xc1
                for eh in range(2):
                    pss = [psA.tile([P, 512], f32, name=f"psq{c}_{eh}_{e4}", tag="psA")
                           for e4 in range(4)]
                    for d_ in range(NE):
                        for e4 in range(4):
                            e = eh * 4 + e4
                            nc.tensor.matmul(
                                pss[e4], mq[:, d_, e * P:(e + 1) * P], xc[:, d_, :],
                                start=(d_ == 0), stop=(d_ == NE - 1),
                            )
                    for e4 in range(4):
                        e = eh * 4 + e4
                        nc.vector.tensor_copy(
                            zT[:, e, c * 512:(c + 1) * 512], pss[e4]
                        )

            # proj weights (needed last; scalar queue keeps sync free for
            # the output stores)
            w2 = wpool.tile([P, NE, D], bf16, name="w2", tag="w")
            for m in range(NE):
                nc.scalar.dma_start(w2[:, m, :], w2_h[m * P:(m + 1) * P, :])

            # ---- attention per sq-chunk ----
            for c, Kc in CHUNKS:
                # scoresT[j] = xT-tiles (stationary) x zT slice; exp(+b_j) -> bf16
                exps = []
                for j in range(Kc):
                    s_off = _chunk_start(c, j) * P
                    Nj = 512 - s_off
                    ps = psA.tile([P, 512], f32, name=f"pss{c}_{j}", tag="psA")
                    psv = ps[:, :Nj]
                    for e in range(NE):
                        nc.tensor.matmul(
                            psv, xT_sb[:, e, j * P:(j + 1) * P],
                            zT[:, e, c * 512 + s_off:(c + 1) * 512],
                            start=(e == 0), stop=(e == NE - 1),
                        )
                    ex = epool.tile([P, 512], bf16, name=f"exp{c}_{j}", tag="exp")
                    exv = ex[:, :Nj]
                    nc.scalar.activation(
                        exv, psv, AF.Exp, bias=bj_sb[:, j:j + 1], scale=1.0
                    )
                    # mask the first sq-subtile where this j is diagonal/padding
                    mi = [i for i in range(4)
                          if PADK[c * 4 + i] - 2 == j or PADK[c * 4 + i] - 1 == j]
                    if mi:
                        i = mi[0]
                        which = int(PADK[c * 4 + i] - 1 == j)
                        gidx = 2 * (c * 4 + i) + which
                        nc.vector.copy_predicated(
                            ex[:, :P], mk_sb[:, gidx, :], zeros_pp
                        )
                    exps.append((exv, s_off, Nj))

                if denom2:
                    # denom[1, sq]: ones (stationary) x exp tiles, j-accumulated
                    # in one PSUM row; then 1/x, consumed partition-broadcast
                    # (stride-0) by the AV evacuation tensor_tensor.
                    pd = psB.tile([1, 512], f32, name=f"dn{c}", tag="psB")
                    for j in range(Kc):
                        exv, s_off, _ = exps[j]
                        nc.tensor.matmul(
                            pd[:, s_off:512], ones_col, exv,
                            start=(j == 0), stop=(j == Kc - 1),
                        )
                    rc = rpool.tile([1, 512], f32, name=f"rc{c}", tag="rc")
                    nc.vector.reciprocal(rc, pd)
                    rb = rbp.tile([P, 512], f32, name=f"rb{c}", tag="rb")
                    nc.gpsimd.partition_broadcast(rb, rc)
                    recips = None
                else:
                    # denom^T[sq,1] per sq-subtile: exp-tiles (stationary) x ones
                    recips = []
                    for s4 in range(4):
                        Pi = PADK[c * 4 + s4]
                        pd = psB.tile([P, 1], f32, name=f"dn{c}_{s4}", tag="psB")
                        for j in range(Pi):
                            exv, s_off, _ = exps[j]
                            nc.tensor.matmul(
                                pd, exv[:, s4 * P - s_off:(s4 + 1) * P - s_off], ones_col,
                                start=(j == 0), stop=(j == Pi - 1),
                            )
                        rc = rpool.tile([P, 1], f32, name=f"rc{c}_{s4}", tag="rc")
                        nc.vector.reciprocal(rc, pd)
                        recips.append(rc)

                # AV: aoT[e, sq] accumulated over sk-tiles; xN tiles stationary
                ao = apool.tile([P, NE, 512], bf16, name=f"ao{c}", tag="ao")
                for m in range(NE):
                    pa = psA.tile([P, 512], f32, name=f"pa{c}_{m}", tag="psA")
                    for j in range(Kc):
                        exv, s_off, _ = exps[j]
                        nc.tensor.matmul(
                            pa[:, s_off:512], xN_sb[:, j, m * P:(m + 1) * P], exv,
                            start=(j == 0), stop=(j == Kc - 1),
                        )
                    if denom2:
                        nc.vector.tensor_tensor(
                            ao[:, m, :], pa, rb, op=mybir.AluOpType.mult
                        )
                    else:
                        nc.vector.tensor_copy(ao[:, m, :], pa)

                # proj: out[sq, e'] = aoT-tiles (stationary) x W2^T; normalize+bias
                for s4 in range(4):
                    otw = (opool.tile([P, D], f32, name=f"otw{c}_{s4}", tag="ot")
                           if obat else None)
                    for n in range(2):
                        po = psA.tile([P, 512], f32, name=f"po{c}_{s4}_{n}", tag="psA")
                        for m in range(NE):
                            nc.tensor.matmul(
                                po, ao[:, m, s4 * P:(s4 + 1) * P],
                                w2[:, m, n * 512:(n + 1) * 512],
                                start=(m == 0), stop=(m == NE - 1),
                            )
                        ot = (otw[:, n * 512:(n + 1) * 512] if obat else
                              opool.tile([P, 512], f32, name=f"ot{c}_{s4}_{n}", tag="ot"))
                        if denom2:
                            nc.vector.tensor_tensor(
                                ot, po, bp_sb[:, n * 512:(n + 1) * 512],
                                op=mybir.AluOpType.add,
                            )
                        else:
                            nc.vector.scalar_tensor_tensor(
                                ot, po, recips[s4], bp_sb[:, n * 512:(n + 1) * 512],
                                op0=mybir.AluOpType.mult, op1=mybir.AluOpType.add,
                            )
                        if not obat:
                            oeng = (nc.scalar if (outq and (s4 * 2 + n) % 2)
                                    else nc.sync)
                            oeng.dma_start(
                                out_h[c * 512 + s4 * P:c * 512 + (s4 + 1) * P,
                                      n * 512:(n + 1) * 512],
                                ot,
                            )
                    if obat:
                        nc.sync.dma_start(
                            out_h[c * 512 + s4 * P:c * 512 + (s4 + 1) * P, :], otw
                        )
        if rep_cm is not None:
            rep_cm.__exit__(None, None, None)
    nc.finalize()
    return nc


_NC_CACHE = None


def _get_nc():
    global _NC_CACHE
    if _NC_CACHE is None:
        _NC_CACHE = build_nc()
    return _NC_CACHE


def _prep_inputs(x, Wq, bq, Wk, bk, Wv, bv, Wp, bp):
    """Host-side shard prep: returns list of per-core input dicts."""
    x = np.asarray(x, np.float32)
    Wq, bq = np.asarray(Wq, np.float32), np.asarray(bq, np.float32)
    Wk, bk = np.asarray(Wk, np.float32), np.asarray(bk, np.float32)
    Wv, bv = np.asarray(Wv, np.float32), np.asarray(bv, np.float32)
    Wp, bp = np.asarray(Wp, np.float32), np.asarray(bp, np.float32)

    u = (Wk.T @ bq) * SCALE                      # per-key bias direction
    shared = {
        "mQ": np.ascontiguousarray((Wq.T @ Wk) * SCALE).astype(nbf),
        "w2T": np.ascontiguousarray((Wp @ Wv).T).astype(nbf),
        "bp_bc": np.ascontiguousarray(
            np.tile((bp + Wp @ bv).astype(np.float32), (P, 1))
        ),
    }

    kk = np.arange(P)[:, None]
    qq = np.arange(P)[None, :]
    in_maps = []
    per_batch = {}
    for b in range(B):
        xb = x[b]
        per_batch[b] = {
            "xT": np.ascontiguousarray(xb.T).astype(nbf),
            "xN": np.ascontiguousarray(xb).astype(nbf),
            "bjT": np.ascontiguousarray((xb @ u).reshape(NSK, P).T.astype(np.float32)),
        }
    for c in range(NCORES):
        b, p = divmod(c, 2)
        g = GT[p]
        xb = x[b]
        qrows = np.concatenate([xb[t * P:(t + 1) * P] for t in g], 0)
        xqT = np.ascontiguousarray(qrows.T).astype(nbf)
        masks = np.zeros((16, P, P), np.float32)
        for i in range(8):
            Pi, gi = PADK[i], g[i]
            for w, j in ((0, Pi - 2), (1, Pi - 1)):
                # 1.0 where DISALLOWED (key index > query index)
                masks[2 * i + w] = ((j * P + kk) > (gi * P + qq)).astype(np.float32)
        in_maps.append({
            "xqT": xqT, "masks": masks.astype(np.uint8),
            **per_batch[b], **shared,
        })
    return in_maps


def _scatter_outputs(results):
    out = np.empty((B, S, D), np.float32)
    for c in range(NCORES):
        b, p = divmod(c, 2)
        o = results[c]["out"]
        for i, t in enumerate(GT[p]):
            out[b, t * P:(t + 1) * P] = o[i * P:(i + 1) * P]
    return out


def run(inputs, trace=False):
    nc = _get_nc()
    in_maps = _prep_inputs(**inputs)
    res = run_bass_kernel_spmd(
        nc, in_maps, core_ids=list(range(NCORES)), trace=trace
    )
    return _scatter_outputs(res.results), res


def kernel(**inputs):
    out, _ = run(inputs)
    return out
